# revision 1
# baseline (speedup 1.0000x reference)
"""Trainium2 Bass kernel for nn_AttentionBlock (B=4, C=256, H=W=64, 4 heads,
GroupNorm(16) + qkv 1x1 + attention + proj 1x1 + residual).

Sharding: 16 (batch, head) units across 8 cores -> 2 heads (same batch) per
core. Each core computes GroupNorm + qkv for its batch (replicated across the
2 cores sharing a batch), attention for its 2 heads, and a partial proj over
its 128 output-side channels. Host sums the two partials per batch.

All matmuls run as float32r (full-rate fp32, ~2^-13 rounding).
"""
import os
import numpy as np
import ml_dtypes
from contextlib import ExitStack

import concourse.bass as bass
import concourse.bacc as bacc
import concourse.tile as tile
from concourse import mybir
from concourse.bass_utils import run_bass_kernel_spmd

F32 = mybir.dt.float32
F32R = mybir.dt.float32r
BF16 = mybir.dt.bfloat16

B, C, HH, WW = 4, 256, 64, 64
T = HH * WW          # 4096
NHEAD = 4            # heads per batch (2 per core)
CH = 64              # channels per head
EPS = 1e-5
SCALE2 = 0.125       # 1/sqrt(ch) applied inside exp
N_CORES = 8
TC = 1024            # attention t-chunk
NST = T // 128       # 32 s-tiles
NTC = T // TC        # 4 t-chunks


def _emit(tc_ctx):
    nc = tc_ctx.nc
    tc = tc_ctx

    d_xb = nc.dram_tensor("xb", [2, 128, T], F32, kind="ExternalInput").ap()
    d_wqk = nc.dram_tensor("wqk", [2, 128, 256], BF16, kind="ExternalInput").ap()
    d_bqk = nc.dram_tensor("bqk", [128, 2], F32, kind="ExternalInput").ap()
    d_wv = nc.dram_tensor("wv", [2, 128, 128], BF16, kind="ExternalInput").ap()
    d_bv = nc.dram_tensor("bv", [1, 128], BF16, kind="ExternalInput").ap()
    d_gnw = nc.dram_tensor("gnw", [128, 2], F32, kind="ExternalInput").ap()
    d_gnb = nc.dram_tensor("gnb", [128, 2], F32, kind="ExternalInput").ap()
    d_gagg = nc.dram_tensor("gagg", [128, 128], F32R, kind="ExternalInput").ap()
    d_pw = nc.dram_tensor("pw", [2, 64, 256], F32R, kind="ExternalInput").ap()
    d_pb = nc.dram_tensor("pb", [128, 2], F32, kind="ExternalInput").ap()
    d_cones = nc.dram_tensor("cones", [128, 128], F32R, kind="ExternalInput").ap()
    d_conesb = nc.dram_tensor("conesb", [128, 128], BF16, kind="ExternalInput").ap()
    d_out = nc.dram_tensor("out", [2, 128, T], F32, kind="ExternalOutput").ap()

    with ExitStack() as ctx:
        persist = ctx.enter_context(tc.tile_pool(name="persist", bufs=1))
        big = ctx.enter_context(tc.tile_pool(name="big", bufs=2))
        small = ctx.enter_context(tc.tile_pool(name="small", bufs=1))

        # ---- persistent SBUF ----
        xb_sb = [persist.tile([128, T], F32, tag=f"xb{i}", name=f"xb_sb{i}") for i in range(2)]
        qk_sb = [persist.tile([128, T], BF16, tag=f"qk{i}", name=f"qk_sb{i}") for i in range(2)]  # [0]=q both heads, [1]=k both heads
        vt2 = persist.tile([128, NST * 130], BF16, tag="vt2")


        w_wqk = small.tile([128, 2, 256], BF16, tag="wqk")
        w_wv = small.tile([128, 2, 128], BF16, tag="wv")
        onesb = small.tile([1, 128], BF16, tag="onesb")
        w_gagg = small.tile([128, 128], F32R, tag="gagg")
        w_pw = [small.tile([64, 256], F32R, tag=f"pw{i}", name=f"w_pw{i}")
                for i in range(2)]
        b_qk = small.tile([128, 2], F32, tag="bqk")
        b_v = small.tile([1, 128], BF16, tag="bv")
        b_gnw = small.tile([128, 2], F32, tag="gnw")
        b_gnb = small.tile([128, 2], F32, tag="gnb")
        b_pb = small.tile([128, 2], F32, tag="pb")
        t_eps = small.tile([128, 1], F32, tag="eps")
        ones_sb = small.tile([128, 128], F32R, tag="ones")

        vt2v = vt2[:].rearrange("p (c h s) -> p c h s", c=NST, h=2, s=65)

        normed = []
        normedb = []

        # ================= S1: load x + GroupNorm =================
        with tc.tile_pool(name="gn_ps", bufs=2, space="PSUM") as gn_ps, \
             tc.tile_pool(name="gn_tmp", bufs=4) as gn_tmp:
            for ct in range(2):
                for sub in range(8):
                    eng = (nc.sync, nc.gpsimd, nc.scalar, nc.gpsimd)[sub % 4]
                    eng.dma_start(xb_sb[ct][:, sub * 512:(sub + 1) * 512],
                                  d_xb[ct, :, sub * 512:(sub + 1) * 512])
            # weights + constants after the latency-critical xb load;
            # the many-descriptor vt2-ones DMA goes to the idle ACT queue
            nc.gpsimd.dma_start(
                vt2v[:, :, :, 64:65],
                d_conesb[:, 0:64].rearrange("p (c h u) -> p c h u",
                                            c=NST, h=2, u=1))
            nc.sync.dma_start(w_wqk[:], d_wqk.rearrange("k c o -> c k o"))
            nc.sync.dma_start(w_wv[:], d_wv.rearrange("k c o -> c k o"))
            nc.sync.dma_start(w_gagg[:], d_gagg)
            nc.sync.dma_start(onesb[:], d_conesb[0:1, :])
            nc.sync.dma_start(ones_sb[:], d_cones)
            for i in range(2):
                nc.sync.dma_start(w_pw[i][:], d_pw[i])
            nc.sync.dma_start(b_qk[:], d_bqk)
            nc.sync.dma_start(b_v[:], d_bv)
            nc.sync.dma_start(b_gnw[:], d_gnw)
            nc.sync.dma_start(b_gnb[:], d_gnb)
            nc.sync.dma_start(b_pb[:], d_pb)
            nc.vector.memset(t_eps[:], EPS / 4)
            for ct in range(2):
                xt = xb_sb[ct]
                stats = gn_tmp.tile([128, 8, 6], F32, tag="stats")
                xv = xt[:].rearrange("p (n f) -> p n f", f=512)
                for sub in range(8):
                    nc.vector.bn_stats(stats[:, sub, :], xv[:, sub, :])
                mv = gn_tmp.tile([128, 2], F32, tag="mv")
                nc.vector.bn_aggr(mv[:], stats[:])
                # stats_in = [mean, var + mean^2] (f32r for the agg matmul)
                sin = gn_tmp.tile([128, 2], F32R, tag="sin")
                msq = gn_tmp.tile([128, 1], F32, tag="msq")
                nc.vector.tensor_mul(msq[:], mv[:, 0:1], mv[:, 0:1])
                nc.vector.tensor_copy(sin[:, 0:1], mv[:, 0:1])
                nc.vector.tensor_add(sin[:, 1:2], mv[:, 1:2], msq[:])
                ps_g = gn_ps.tile([128, 2], F32, tag="gps")
                nc.tensor.matmul(ps_g[:], w_gagg[:], sin[:], start=True, stop=True)
                g_sb = gn_tmp.tile([128, 2], F32, tag="gsb")
                nc.vector.tensor_copy(g_sb[:], ps_g[:])
                gm2 = gn_tmp.tile([128, 1], F32, tag="gm2")
                nc.vector.tensor_mul(gm2[:], g_sb[:, 0:1], g_sb[:, 0:1])
                gvar = gn_tmp.tile([128, 1], F32, tag="gvar")
                nc.vector.tensor_sub(gvar[:], g_sb[:, 1:2], gm2[:])
                srt = gn_tmp.tile([128, 1], F32, tag="srt")
                nc.scalar.activation(srt[:], gvar[:],
                                     mybir.ActivationFunctionType.Sqrt,
                                     bias=t_eps[:], scale=1.0)
                rstd = gn_tmp.tile([128, 1], F32, tag="rstd")
                nc.vector.reciprocal(rstd[:], srt[:])
                # fold (x-gm)*rstd*w + b into one ACT pass: x*sc + bi with
                # sc = rstd*w, bi = b - gm*rstd*w  (per-partition scalars)
                sc = gn_tmp.tile([128, 1], F32, tag="sc")
                nc.vector.tensor_mul(sc[:], rstd[:], b_gnw[:, ct:ct + 1])
                bi = gn_tmp.tile([128, 1], F32, tag="bi")
                nc.vector.tensor_mul(bi[:], g_sb[:, 0:1], sc[:])
                nc.vector.tensor_sub(bi[:], b_gnb[:, ct:ct + 1], bi[:])
                cb = persist.tile([128, T], BF16, tag=f"nb{ct}",
                                  name=f"normedb{ct}")
                nc.scalar.activation(cb[:], xt[:],
                                     mybir.ActivationFunctionType.Identity,
                                     bias=bi[:], scale=sc[:])
                normedb.append(cb)

        # ================= S2: qkv + v transpose =================
        with tc.tile_pool(name="qkv_ps", bufs=3, space="PSUM") as qkv_ps, \
             tc.tile_pool(name="vt_ps", bufs=2, space="PSUM") as vt_ps:
            for ot in range(2):
                for chk in range(8):
                    ps = qkv_ps.tile([128, 512], F32, tag="qkv")
                    for kt in range(2):
                        nc.tensor.matmul(
                            ps[:], w_wqk[:, kt, ot * 128:(ot + 1) * 128],
                            normedb[kt][:, chk * 512:(chk + 1) * 512],
                            start=(kt == 0), stop=(kt == 1))
                    nc.vector.tensor_scalar(
                        out=qk_sb[ot][:, chk * 512:(chk + 1) * 512], in0=ps[:],
                        scalar1=b_qk[:, ot:ot + 1], scalar2=None,
                        op0=mybir.AluOpType.add)
            # vT computed directly: out[s, c] = normed[:, s].T @ Wv[:, c],
            # bias bv added via a ones-row K=1 accumulate
            for chk in range(NST):
                pvt = vt_ps.tile([128, 128], F32, tag="vt")
                for kt in range(2):
                    nc.tensor.matmul(
                        pvt[:], normedb[kt][:, chk * 128:(chk + 1) * 128],
                        w_wv[:, kt, :], start=(kt == 0), stop=False)
                nc.tensor.matmul(
                    pvt[:], onesb[:], b_v[:],
                    start=False, stop=True)
                nc.scalar.copy(
                    vt2v[:, chk, :, 0:64],
                    pvt[:].rearrange("p (h s) -> p h s", h=2))

        # ================= S3: attention =================
        stage_pool = ctx.enter_context(tc.tile_pool(name="stage", bufs=8))
        den_pool = ctx.enter_context(tc.tile_pool(name="den", bufs=2))
        bcr_pool2 = ctx.enter_context(tc.tile_pool(name="bcr2", bufs=2))
        stages = {}
        with tc.tile_pool(name="qk_ps", bufs=2, space="PSUM") as qk_ps, \
             tc.tile_pool(name="av_ps", bufs=2, space="PSUM") as av_ps, \
             tc.tile_pool(name="exp_sb", bufs=9) as exp_pool:
            for tci in range(NTC):
                avs = [av_ps.tile([65, TC], F32, tag="av", name=f"av{tci}_{hh}") for hh in range(2)]
                # software pipeline: AV lags QK by one j so its exp wait
                # is pre-satisfied (an unsatisfied wait blocks the PE queue's
                # LDWEIGHTS pull-ahead and serializes the weight loads)
                pend = []
                for j in range(NST):
                    for h in range(2):
                        qs = qk_ps.tile([128, TC], F32, tag="qk")
                        for n2 in range(TC // 512):
                            nc.tensor.matmul(
                                qs[:, n2 * 512:(n2 + 1) * 512],
                                qk_sb[1][h * 64:(h + 1) * 64, j * 128:(j + 1) * 128],
                                qk_sb[0][h * 64:(h + 1) * 64,
                                         tci * TC + n2 * 512: tci * TC + (n2 + 1) * 512],
                                start=True, stop=True)
                        e = exp_pool.tile([128, TC], BF16, tag="exp")
                        nc.scalar.activation(e[:], qs[:],
                                             mybir.ActivationFunctionType.Exp,
                                             scale=SCALE2)
                        pend.append((j, h, e))
                    while len(pend) > 4:
                        pj, ph, pe = pend.pop(0)
                        for n2 in range(TC // 512):
                            nc.tensor.matmul(
                                avs[ph][:, n2 * 512:(n2 + 1) * 512],
                                vt2v[:, pj, ph, :], pe[:, n2 * 512:(n2 + 1) * 512],
                                start=(pj == 0), stop=(pj == NST - 1),
                                skip_group_check=True)
                for pj, ph, pe in pend:
                    for n2 in range(TC // 512):
                        nc.tensor.matmul(
                            avs[ph][:, n2 * 512:(n2 + 1) * 512],
                            vt2v[:, pj, ph, :], pe[:, n2 * 512:(n2 + 1) * 512],
                            start=(pj == 0), stop=(pj == NST - 1),
                            skip_group_check=True)
                for h in range(2):
                    st = stage_pool.tile([65, TC], F32R, tag="stage",
                                         name=f"st{tci}_{h}")
                    nc.vector.tensor_copy(st[:], avs[h][:])
                    stages[(h, tci)] = st
                    # normalize in-loop: denom row -> partition 0 (DMA),
                    # reciprocal (DVE), broadcast (GpSimd), scale (DVE) --
                    # no PE/PSUM involved, so it overlaps the attention loop
                    den0 = den_pool.tile([1, TC], F32, tag="den0",
                                         name=f"dn{tci}_{h}")
                    nc.gpsimd.dma_start(den0[:], st[64:65, :])
                    denR = den_pool.tile([1, TC], F32, tag="denR",
                                         name=f"dr{tci}_{h}")
                    nc.vector.reciprocal_approx_fast(denR[:], den0[:])
                    bcr2 = bcr_pool2.tile([64, TC], F32, tag="bcr2",
                                          name=f"bc{tci}_{h}")
                    nc.gpsimd.partition_broadcast(bcr2[:], denR[:], channels=64)
                    nc.vector.tensor_mul(st[0:64, :], st[0:64, :], bcr2[:])

        # ================= S4: normalize + proj + residual =================
        with tc.tile_pool(name="pj_ps", bufs=3, space="PSUM") as pj_ps, \
             tc.tile_pool(name="osb", bufs=6) as osb_pool:
            for tci in range(NTC):
                for chk in (2 * tci, 2 * tci + 1):
                    for ot in range(2):
                        ps = pj_ps.tile([128, 512], F32, tag="pj")
                        for h in range(2):
                            nc.tensor.matmul(
                                ps[:], w_pw[h][:, ot * 128:(ot + 1) * 128],
                                stages[(h, tci)][0:64, (chk % 2) * 512:
                                                 (chk % 2 + 1) * 512],
                                start=(h == 0), stop=(h == 1))
                        osb = osb_pool.tile([128, 512], F32, tag="osb")
                        nc.scalar.activation(
                            osb[:], ps[:],
                            mybir.ActivationFunctionType.Identity,
                            bias=b_pb[:, ot:ot + 1], scale=1.0)
                        nc.vector.tensor_add(
                            osb[:], osb[:],
                            xb_sb[ot][:, chk * 512:(chk + 1) * 512])
                        (nc.sync if ot == 0 else nc.gpsimd).dma_start(
                            d_out[ot, :, chk * 512:(chk + 1) * 512], osb[:])


_NC_CACHE = None


def build_nc():
    global _NC_CACHE
    if _NC_CACHE is not None:
        return _NC_CACHE
    nc = bacc.Bacc("TRN2", target_bir_lowering=False, debug=False,
                   num_devices=N_CORES)
    with tile.TileContext(nc) as t:
        _emit(t)
    nc.compile()
    _NC_CACHE = nc
    return nc


def make_core_inputs(inputs, core):
    x = np.ascontiguousarray(np.asarray(inputs["x"], np.float32))
    norm_w = np.asarray(inputs["norm_w"], np.float32)
    norm_b = np.asarray(inputs["norm_b"], np.float32)
    qkv_w = np.asarray(inputs["qkv_w"], np.float32)
    qkv_b = np.asarray(inputs["qkv_b"], np.float32)
    proj_w = np.asarray(inputs["proj_w"], np.float32)
    proj_b = np.asarray(inputs["proj_b"], np.float32)
    b, p = core // 2, core % 2
    ha, hb = 2 * p, 2 * p + 1
    x2 = x.reshape(B, C, T)

    def rows(h, part):
        base = 192 * h + 64 * part
        return slice(base, base + 64)

    xb = np.ascontiguousarray((0.5 * x2[b]).reshape(2, 128, T))
    # o-tile 0 = [q_ha, q_hb], o-tile 1 = [k_ha, k_hb] (per-head slices of the
    # q / k SBUF tiles then share a base partition, which matmul requires)
    wqk_rows = np.concatenate([qkv_w[rows(ha, 0)], qkv_w[rows(hb, 0)],
                               qkv_w[rows(ha, 1)], qkv_w[rows(hb, 1)]], axis=0)
    wqk = np.ascontiguousarray(wqk_rows.T.reshape(2, 128, 256)).astype(ml_dtypes.bfloat16)
    bqk = np.ascontiguousarray(
        np.concatenate([qkv_b[rows(ha, 0)], qkv_b[rows(hb, 0)],
                        qkv_b[rows(ha, 1)], qkv_b[rows(hb, 1)]]).reshape(2, 128).T)
    wv_rows = np.concatenate([qkv_w[rows(ha, 2)], qkv_w[rows(hb, 2)]], axis=0)
    wv = np.ascontiguousarray(wv_rows.T.reshape(2, 128, 128)).astype(ml_dtypes.bfloat16)
    bv = np.ascontiguousarray(
        np.concatenate([qkv_b[rows(ha, 2)],
                        qkv_b[rows(hb, 2)]]).reshape(1, 128)).astype(ml_dtypes.bfloat16)
    gnw = np.ascontiguousarray(norm_w.reshape(2, 128).T)
    gnb = np.ascontiguousarray(norm_b.reshape(2, 128).T)
    gagg = np.kron(np.eye(8, dtype=np.float32),
                   np.ones((16, 16), np.float32) / 16.0)
    pw = np.ascontiguousarray(
        proj_w[:, 128 * p:128 * p + 128].T.reshape(2, 64, 256))
    pb = np.ascontiguousarray((0.5 * proj_b).reshape(2, 128).T)
    cones = np.ones((128, 128), np.float32)
    conesb = np.ones((128, 128), ml_dtypes.bfloat16)
    return dict(xb=xb, wqk=wqk, bqk=bqk, wv=wv, bv=bv, gnw=gnw, gnb=gnb,
                gagg=gagg, pw=pw, pb=pb, cones=cones, conesb=conesb)


def _ensure_axon_devices():
    """The SPMD run needs the 8 axon-tunneled NeuronCores visible to jax.
    If a caller pinned jax to cpu (e.g. to run the reference), try to undo."""
    import jax
    try:
        if len(jax.devices("axon")) >= N_CORES:
            return
    except Exception:
        pass
    try:
        os.environ.pop("JAX_PLATFORMS", None)
        jax.config.update("jax_platforms", None)
        jax.extend.backend.clear_backends()
    except Exception:
        pass


def kernel(**inputs):
    try:
        import jax
        if not any(d.platform == "axon" for d in jax.devices()):
            _ensure_axon_devices()
    except Exception:
        _ensure_axon_devices()
    nc = build_nc()
    in_maps = [make_core_inputs(inputs, core) for core in range(N_CORES)]
    res = None
    last_err = None
    for attempt in range(4):
        try:
            res = run_bass_kernel_spmd(nc, in_maps, list(range(N_CORES)))
            break
        except Exception as e:  # transient NRT_EXEC_UNIT_UNRECOVERABLE etc.
            last_err = e
            import time as _time
            _time.sleep(2.0)
    if res is None:
        raise last_err
    out = np.empty((B, C, T), np.float32)
    for b in range(B):
        out[b] = (res.results[2 * b]["out"].reshape(C, T)
                  + res.results[2 * b + 1]["out"].reshape(C, T))
    return out.reshape(B, C, HH, WW)



# revision 5
# speedup vs baseline: 1.2123x; 1.2123x over previous
"""Trainium2 Bass kernel for nn_AttentionBlock (B=4, C=256, H=W=64, 4 heads,
GroupNorm(16) + qkv 1x1 + attention + proj 1x1 + residual).

Sharding: 16 (batch, head) units across 8 cores -> 2 heads (same batch) per
core. Each core computes GroupNorm + qkv for its batch (replicated across the
2 cores sharing a batch), attention for its 2 heads, and a partial proj over
its 128 output-side channels. Host sums the two partials per batch.

All matmuls run as float32r (full-rate fp32, ~2^-13 rounding).
"""
import os
import numpy as np
import ml_dtypes
from contextlib import ExitStack

import concourse.bass as bass
import concourse.bacc as bacc
import concourse.tile as tile
from concourse import mybir
from concourse.bass_utils import run_bass_kernel_spmd

F32 = mybir.dt.float32
F32R = mybir.dt.float32r
BF16 = mybir.dt.bfloat16
I16 = mybir.dt.int16

B, C, HH, WW = 4, 256, 64, 64
T = HH * WW          # 4096
NHEAD = 4            # heads per batch (2 per core)
CH = 64              # channels per head
EPS = 1e-5
SCALE2 = 0.125       # 1/sqrt(ch) applied inside exp
N_CORES = 8
TC = 512             # attention t-chunk (1 PSUM bank)
NST = T // 128       # 32 s-tiles
NTC = T // TC        # 8 t-chunks
# Schraudolph-style exp for the DVE path: bf16 bits of exp(SCALE2*x) are
# approximated by round(x*SCH_A + SCH_B) computed as f32 -> int16 convert,
# then the int16 tile is bitcast to bf16. Max per-element rel err ~3.3%,
# which washes out through the softmax normalization (verified 7.6e-4
# end-to-end with ALL tiles approximated).
SCH_A = 23.083120654223414   # SCALE2 * log2(e) * 128
SCH_B = 16250.4


def _emit(tc_ctx):
    nc = tc_ctx.nc
    tc = tc_ctx

    d_xb = nc.dram_tensor("xb", [2, 128, T], F32, kind="ExternalInput").ap()
    d_wqk = nc.dram_tensor("wqk", [2, 128, 256], BF16, kind="ExternalInput").ap()
    d_bqk = nc.dram_tensor("bqk", [128, 2], F32, kind="ExternalInput").ap()
    d_wv = nc.dram_tensor("wv", [2, 128, 128], BF16, kind="ExternalInput").ap()
    d_bv = nc.dram_tensor("bv", [1, 128], BF16, kind="ExternalInput").ap()
    d_gnw = nc.dram_tensor("gnw", [128, 2], F32, kind="ExternalInput").ap()
    d_gnb = nc.dram_tensor("gnb", [128, 2], F32, kind="ExternalInput").ap()
    d_gagg = nc.dram_tensor("gagg", [128, 128], F32R, kind="ExternalInput").ap()
    d_pw = nc.dram_tensor("pw", [2, 64, 256], F32R, kind="ExternalInput").ap()
    d_pb = nc.dram_tensor("pb", [128, 2], F32, kind="ExternalInput").ap()
    d_cones = nc.dram_tensor("cones", [128, 128], F32R, kind="ExternalInput").ap()
    d_conesb = nc.dram_tensor("conesb", [128, 128], BF16, kind="ExternalInput").ap()
    d_out = nc.dram_tensor("out", [2, 128, T], F32, kind="ExternalOutput").ap()

    with ExitStack() as ctx:
        persist = ctx.enter_context(tc.tile_pool(name="persist", bufs=1))
        big = ctx.enter_context(tc.tile_pool(name="big", bufs=2))
        small = ctx.enter_context(tc.tile_pool(name="small", bufs=1))

        # ---- persistent SBUF ----
        xb_sb = [persist.tile([128, T], F32, tag=f"xb{i}", name=f"xb_sb{i}") for i in range(2)]
        qk_sb = [persist.tile([128, T], BF16, tag=f"qk{i}", name=f"qk_sb{i}") for i in range(2)]  # [0]=q both heads, [1]=k both heads
        vt2 = persist.tile([128, NST * 130], BF16, tag="vt2")


        w_wqk = small.tile([128, 2, 256], BF16, tag="wqk")
        w_wv = small.tile([128, 2, 128], BF16, tag="wv")
        onesb = small.tile([1, 128], BF16, tag="onesb")
        w_gagg = small.tile([128, 128], F32R, tag="gagg")
        w_pw = [small.tile([64, 256], F32R, tag=f"pw{i}", name=f"w_pw{i}")
                for i in range(2)]
        b_qk = small.tile([128, 2], F32, tag="bqk")
        b_v = small.tile([1, 128], BF16, tag="bv")
        b_gnw = small.tile([128, 2], F32, tag="gnw")
        b_gnb = small.tile([128, 2], F32, tag="gnb")
        b_pb = small.tile([128, 2], F32, tag="pb")
        t_eps = small.tile([128, 1], F32, tag="eps")
        ones_sb = small.tile([128, 128], F32R, tag="ones")

        vt2v = vt2[:].rearrange("p (c h s) -> p c h s", c=NST, h=2, s=65)

        normed = []
        normedb = []

        # ================= S1: load x + GroupNorm =================
        with tc.tile_pool(name="gn_ps", bufs=2, space="PSUM") as gn_ps, \
             tc.tile_pool(name="gn_tmp", bufs=4) as gn_tmp:
            for ct in range(2):
                for sub in range(8):
                    eng = (nc.sync, nc.gpsimd, nc.scalar, nc.gpsimd)[sub % 4]
                    eng.dma_start(xb_sb[ct][:, sub * 512:(sub + 1) * 512],
                                  d_xb[ct, :, sub * 512:(sub + 1) * 512])
            # weights + constants after the latency-critical xb load;
            # the many-descriptor vt2-ones DMA goes to the idle ACT queue
            nc.gpsimd.dma_start(
                vt2v[:, :, :, 64:65],
                d_conesb[:, 0:64].rearrange("p (c h u) -> p c h u",
                                            c=NST, h=2, u=1))
            nc.sync.dma_start(w_wqk[:], d_wqk.rearrange("k c o -> c k o"))
            nc.sync.dma_start(w_wv[:], d_wv.rearrange("k c o -> c k o"))
            nc.sync.dma_start(w_gagg[:], d_gagg)
            nc.sync.dma_start(onesb[:], d_conesb[0:1, :])
            nc.sync.dma_start(ones_sb[:], d_cones)
            for i in range(2):
                nc.sync.dma_start(w_pw[i][:], d_pw[i])
            nc.sync.dma_start(b_qk[:], d_bqk)
            nc.sync.dma_start(b_v[:], d_bv)
            nc.sync.dma_start(b_gnw[:], d_gnw)
            nc.sync.dma_start(b_gnb[:], d_gnb)
            nc.sync.dma_start(b_pb[:], d_pb)
            nc.vector.memset(t_eps[:], EPS / 4)
            for ct in range(2):
                xt = xb_sb[ct]
                stats = gn_tmp.tile([128, 8, 6], F32, tag="stats")
                xv = xt[:].rearrange("p (n f) -> p n f", f=512)
                for sub in range(8):
                    nc.vector.bn_stats(stats[:, sub, :], xv[:, sub, :])
                mv = gn_tmp.tile([128, 2], F32, tag="mv")
                nc.vector.bn_aggr(mv[:], stats[:])
                # stats_in = [mean, var + mean^2] (f32r for the agg matmul)
                sin = gn_tmp.tile([128, 2], F32R, tag="sin")
                msq = gn_tmp.tile([128, 1], F32, tag="msq")
                nc.vector.tensor_mul(msq[:], mv[:, 0:1], mv[:, 0:1])
                nc.vector.tensor_copy(sin[:, 0:1], mv[:, 0:1])
                nc.vector.tensor_add(sin[:, 1:2], mv[:, 1:2], msq[:])
                ps_g = gn_ps.tile([128, 2], F32, tag="gps")
                nc.tensor.matmul(ps_g[:], w_gagg[:], sin[:], start=True, stop=True)
                g_sb = gn_tmp.tile([128, 2], F32, tag="gsb")
                nc.vector.tensor_copy(g_sb[:], ps_g[:])
                gm2 = gn_tmp.tile([128, 1], F32, tag="gm2")
                nc.vector.tensor_mul(gm2[:], g_sb[:, 0:1], g_sb[:, 0:1])
                gvar = gn_tmp.tile([128, 1], F32, tag="gvar")
                nc.vector.tensor_sub(gvar[:], g_sb[:, 1:2], gm2[:])
                srt = gn_tmp.tile([128, 1], F32, tag="srt")
                nc.scalar.activation(srt[:], gvar[:],
                                     mybir.ActivationFunctionType.Sqrt,
                                     bias=t_eps[:], scale=1.0)
                rstd = gn_tmp.tile([128, 1], F32, tag="rstd")
                nc.vector.reciprocal(rstd[:], srt[:])
                # fold (x-gm)*rstd*w + b into one ACT pass: x*sc + bi with
                # sc = rstd*w, bi = b - gm*rstd*w  (per-partition scalars)
                sc = gn_tmp.tile([128, 1], F32, tag="sc")
                nc.vector.tensor_mul(sc[:], rstd[:], b_gnw[:, ct:ct + 1])
                bi = gn_tmp.tile([128, 1], F32, tag="bi")
                nc.vector.tensor_mul(bi[:], g_sb[:, 0:1], sc[:])
                nc.vector.tensor_sub(bi[:], b_gnb[:, ct:ct + 1], bi[:])
                cb = persist.tile([128, T], BF16, tag=f"nb{ct}",
                                  name=f"normedb{ct}")
                nc.scalar.activation(cb[:], xt[:],
                                     mybir.ActivationFunctionType.Identity,
                                     bias=bi[:], scale=sc[:])
                normedb.append(cb)

        # ================= S2: qkv + v transpose =================
        with tc.tile_pool(name="qkv_ps", bufs=3, space="PSUM") as qkv_ps, \
             tc.tile_pool(name="vt_ps", bufs=2, space="PSUM") as vt_ps:
            for ot in range(2):
                for chk in range(8):
                    ps = qkv_ps.tile([128, 512], F32, tag="qkv")
                    for kt in range(2):
                        nc.tensor.matmul(
                            ps[:], w_wqk[:, kt, ot * 128:(ot + 1) * 128],
                            normedb[kt][:, chk * 512:(chk + 1) * 512],
                            start=(kt == 0), stop=(kt == 1))
                    nc.vector.tensor_scalar(
                        out=qk_sb[ot][:, chk * 512:(chk + 1) * 512], in0=ps[:],
                        scalar1=b_qk[:, ot:ot + 1], scalar2=None,
                        op0=mybir.AluOpType.add)
            # vT computed directly: out[s, c] = normed[:, s].T @ Wv[:, c],
            # bias bv added via a ones-row K=1 accumulate
            for chk in range(NST):
                pvt = vt_ps.tile([128, 128], F32, tag="vt")
                for kt in range(2):
                    nc.tensor.matmul(
                        pvt[:], normedb[kt][:, chk * 128:(chk + 1) * 128],
                        w_wv[:, kt, :], start=(kt == 0), stop=False)
                nc.tensor.matmul(
                    pvt[:], onesb[:], b_v[:],
                    start=False, stop=True)
                nc.scalar.copy(
                    vt2v[:, chk, :, 0:64],
                    pvt[:].rearrange("p (h s) -> p h s", h=2))

        # ================= S3: attention =================
        # Per s-tile j the two heads' QK matmuls are K=64 and sit in
        # disjoint PE row groups (partitions 0-63 / 64-127), so they run
        # CONCURRENTLY (tile_position auto-derived from base partitions).
        # exp is split across engines: h0 on ACT (true exp), h1 on DVE
        # via the Schraudolph int16 bit-trick -- the two engines run in
        # parallel, halving the softmax bottleneck.
        stage_pool = ctx.enter_context(tc.tile_pool(name="stage", bufs=16))
        den_pool = ctx.enter_context(tc.tile_pool(name="den", bufs=2))
        bcr_pool2 = ctx.enter_context(tc.tile_pool(name="bcr2", bufs=2))
        stages = {}
        with tc.tile_pool(name="qk_ps", bufs=4, space="PSUM") as qk_ps, \
             tc.tile_pool(name="av_ps", bufs=2, space="PSUM") as av_ps, \
             tc.tile_pool(name="exp_sb", bufs=8) as exp_pool:
            for tci in range(NTC):
                avs = [av_ps.tile([65, TC], F32, tag="av", name=f"av{tci}_{hh}") for hh in range(2)]
                # software pipeline: AV lags QK by two j so its exp wait
                # is pre-satisfied (an unsatisfied wait blocks the PE queue's
                # LDWEIGHTS pull-ahead and serializes the weight loads)
                pend = []
                for j in range(NST):
                    qs = [qk_ps.tile([128, TC], F32, tag="qk",
                                     name=f"qs{tci}_{j}_{hh}") for hh in range(2)]
                    for h in range(2):
                        nc.tensor.matmul(
                            qs[h][:],
                            qk_sb[1][h * 64:(h + 1) * 64, j * 128:(j + 1) * 128],
                            qk_sb[0][h * 64:(h + 1) * 64, tci * TC:(tci + 1) * TC],
                            start=True, stop=True)
                    for h in range(2):
                        ei = exp_pool.tile([128, TC], I16, tag="exp",
                                           name=f"e{tci}_{j}_{h}")
                        if h == 0:
                            nc.scalar.activation(ei[:].bitcast(BF16), qs[h][:],
                                                 mybir.ActivationFunctionType.Exp,
                                                 scale=SCALE2)
                        else:
                            nc.vector.tensor_scalar(
                                out=ei[:], in0=qs[h][:],
                                scalar1=SCH_A, scalar2=SCH_B,
                                op0=mybir.AluOpType.mult,
                                op1=mybir.AluOpType.add)
                        pend.append((j, h, ei))
                    while len(pend) > 4:
                        pj, ph, pe = pend.pop(0)
                        nc.tensor.matmul(
                            avs[ph][:], vt2v[:, pj, ph, :],
                            pe[:].bitcast(BF16),
                            start=(pj == 0), stop=(pj == NST - 1),
                            skip_group_check=True)
                for pj, ph, pe in pend:
                    nc.tensor.matmul(
                        avs[ph][:], vt2v[:, pj, ph, :],
                        pe[:].bitcast(BF16),
                        start=(pj == 0), stop=(pj == NST - 1),
                        skip_group_check=True)
                for h in range(2):
                    st = stage_pool.tile([65, TC], F32R, tag="stage",
                                         name=f"st{tci}_{h}")
                    nc.vector.tensor_copy(st[:], avs[h][:])
                    stages[(h, tci)] = st
                    # normalize in-loop: denom row -> partition 0 (DMA),
                    # reciprocal (DVE), broadcast (GpSimd), scale (DVE) --
                    # no PE/PSUM involved, so it overlaps the attention loop
                    den0 = den_pool.tile([1, TC], F32, tag="den0",
                                         name=f"dn{tci}_{h}")
                    nc.gpsimd.dma_start(den0[:], st[64:65, :])
                    denR = den_pool.tile([1, TC], F32, tag="denR",
                                         name=f"dr{tci}_{h}")
                    nc.vector.reciprocal_approx_fast(denR[:], den0[:])
                    bcr2 = bcr_pool2.tile([64, TC], F32, tag="bcr2",
                                          name=f"bc{tci}_{h}")
                    nc.gpsimd.partition_broadcast(bcr2[:], denR[:], channels=64)
                    nc.vector.tensor_mul(st[0:64, :], st[0:64, :], bcr2[:])

        # ================= S4: proj + residual =================
        with tc.tile_pool(name="pj_ps", bufs=3, space="PSUM") as pj_ps, \
             tc.tile_pool(name="osb", bufs=6) as osb_pool:
            for tci in range(NTC):
                for ot in range(2):
                    ps = pj_ps.tile([128, TC], F32, tag="pj")
                    for h in range(2):
                        nc.tensor.matmul(
                            ps[:], w_pw[h][:, ot * 128:(ot + 1) * 128],
                            stages[(h, tci)][0:64, :],
                            start=(h == 0), stop=(h == 1))
                    osb = osb_pool.tile([128, TC], F32, tag="osb")
                    nc.scalar.activation(
                        osb[:], ps[:],
                        mybir.ActivationFunctionType.Identity,
                        bias=b_pb[:, ot:ot + 1], scale=1.0)
                    nc.vector.tensor_add(
                        osb[:], osb[:],
                        xb_sb[ot][:, tci * TC:(tci + 1) * TC])
                    (nc.sync if ot == 0 else nc.gpsimd).dma_start(
                        d_out[ot, :, tci * TC:(tci + 1) * TC], osb[:])


_NC_CACHE = None


def build_nc():
    global _NC_CACHE
    if _NC_CACHE is not None:
        return _NC_CACHE
    nc = bacc.Bacc("TRN2", target_bir_lowering=False, debug=False,
                   num_devices=N_CORES)
    with tile.TileContext(nc) as t:
        _emit(t)
    nc.compile()
    _NC_CACHE = nc
    return nc


def make_core_inputs(inputs, core):
    x = np.ascontiguousarray(np.asarray(inputs["x"], np.float32))
    norm_w = np.asarray(inputs["norm_w"], np.float32)
    norm_b = np.asarray(inputs["norm_b"], np.float32)
    qkv_w = np.asarray(inputs["qkv_w"], np.float32)
    qkv_b = np.asarray(inputs["qkv_b"], np.float32)
    proj_w = np.asarray(inputs["proj_w"], np.float32)
    proj_b = np.asarray(inputs["proj_b"], np.float32)
    b, p = core // 2, core % 2
    ha, hb = 2 * p, 2 * p + 1
    x2 = x.reshape(B, C, T)

    def rows(h, part):
        base = 192 * h + 64 * part
        return slice(base, base + 64)

    xb = np.ascontiguousarray((0.5 * x2[b]).reshape(2, 128, T))
    # o-tile 0 = [q_ha, q_hb], o-tile 1 = [k_ha, k_hb] (per-head slices of the
    # q / k SBUF tiles then share a base partition, which matmul requires)
    wqk_rows = np.concatenate([qkv_w[rows(ha, 0)], qkv_w[rows(hb, 0)],
                               qkv_w[rows(ha, 1)], qkv_w[rows(hb, 1)]], axis=0)
    wqk = np.ascontiguousarray(wqk_rows.T.reshape(2, 128, 256)).astype(ml_dtypes.bfloat16)
    bqk = np.ascontiguousarray(
        np.concatenate([qkv_b[rows(ha, 0)], qkv_b[rows(hb, 0)],
                        qkv_b[rows(ha, 1)], qkv_b[rows(hb, 1)]]).reshape(2, 128).T)
    wv_rows = np.concatenate([qkv_w[rows(ha, 2)], qkv_w[rows(hb, 2)]], axis=0)
    wv = np.ascontiguousarray(wv_rows.T.reshape(2, 128, 128)).astype(ml_dtypes.bfloat16)
    bv = np.ascontiguousarray(
        np.concatenate([qkv_b[rows(ha, 2)],
                        qkv_b[rows(hb, 2)]]).reshape(1, 128)).astype(ml_dtypes.bfloat16)
    gnw = np.ascontiguousarray(norm_w.reshape(2, 128).T)
    gnb = np.ascontiguousarray(norm_b.reshape(2, 128).T)
    gagg = np.kron(np.eye(8, dtype=np.float32),
                   np.ones((16, 16), np.float32) / 16.0)
    pw = np.ascontiguousarray(
        proj_w[:, 128 * p:128 * p + 128].T.reshape(2, 64, 256))
    pb = np.ascontiguousarray((0.5 * proj_b).reshape(2, 128).T)
    cones = np.ones((128, 128), np.float32)
    conesb = np.ones((128, 128), ml_dtypes.bfloat16)
    return dict(xb=xb, wqk=wqk, bqk=bqk, wv=wv, bv=bv, gnw=gnw, gnb=gnb,
                gagg=gagg, pw=pw, pb=pb, cones=cones, conesb=conesb)


def _ensure_axon_devices():
    """The SPMD run needs the 8 axon-tunneled NeuronCores visible to jax.
    If a caller pinned jax to cpu (e.g. to run the reference), try to undo."""
    import jax
    try:
        if len(jax.devices("axon")) >= N_CORES:
            return
    except Exception:
        pass
    try:
        os.environ.pop("JAX_PLATFORMS", None)
        jax.config.update("jax_platforms", None)
        jax.extend.backend.clear_backends()
    except Exception:
        pass


def kernel(**inputs):
    try:
        import jax
        if not any(d.platform == "axon" for d in jax.devices()):
            _ensure_axon_devices()
    except Exception:
        _ensure_axon_devices()
    nc = build_nc()
    in_maps = [make_core_inputs(inputs, core) for core in range(N_CORES)]
    res = None
    last_err = None
    for attempt in range(4):
        try:
            res = run_bass_kernel_spmd(nc, in_maps, list(range(N_CORES)))
            break
        except Exception as e:  # transient NRT_EXEC_UNIT_UNRECOVERABLE etc.
            last_err = e
            import time as _time
            _time.sleep(2.0)
    if res is None:
        raise last_err
    out = np.empty((B, C, T), np.float32)
    for b in range(B):
        out[b] = (res.results[2 * b]["out"].reshape(C, T)
                  + res.results[2 * b + 1]["out"].reshape(C, T))
    return out.reshape(B, C, HH, WW)

